# revision 9
# baseline (speedup 1.0000x reference)
"""Trainium2 Bass kernel for nn_Attention_47261820125787.

Full multi-head attention layer with low-rank-reconstructed projection
weights (w = LM @ RM + W), B=16, S=1024, H=1024, 16 heads x 64 dim.

Sharding: data-parallel over batch -- each of the 8 cores processes 2
batches with the full set of weights. No collectives.

Host-side prep is layout-only: weights are passed pre-transposed
(W.T, LM.T) and hidden_states is passed as [B, H, S] so that every DMA
is contiguous; all math (weight reconstruction, projections, attention)
runs on-device in fp32r.

On-device dataflow per core:
  wT = (LM@RM).T + W.T          reconstructed per projection in SBUF
  qT, kT = [o, s] layouts       (lhsT = wT tiles, rhs = xT tiles)
  v     = [s, o] layout         (lhsT = xT tiles, rhs = wT tiles),
          stored with a ones-column interleaved per head so the ctx
          matmul also produces softmax denominators for free
  E^T   = exp(scores^T / 8)     scores computed transposed [sk, sq];
          no max-subtraction (scores ~ N(0,1), exp can't overflow)
  ctx^T = (v|1).T @ E           [d+1, sq] per head; row 64 = sum_k E
  ctx normalized by 1/sums (vector.reciprocal + gpsimd partition
  broadcast), out = ctx @ wo.T + bo evicted in natural [s, o] layout.
"""

import numpy as np
from contextlib import ExitStack

import concourse.bass as bass
import concourse.tile as tile
from concourse import bacc, mybir
from concourse.bass_utils import run_bass_kernel_spmd

B, S, H, NH, HD = 16, 1024, 1024, 16, 64
KR = 64             # low-rank dim
N_CORES = 8
BPC = B // N_CORES  # batches per core

f32 = mybir.dt.float32
f32r = mybir.dt.float32r
AF = mybir.ActivationFunctionType
ALU = mybir.AluOpType

PROJS = ("q", "k", "v", "o")
NT = H // 128       # 8 partition tiles over hidden dim
NSC = S // 512      # 2 free chunks over sequence
VW = NH * (HD + 1) + 64  # 64 cols + ones col per head + window pad


def _emit(ctx: ExitStack, tc: tile.TileContext, d: dict):
    nc = tc.nc

    # ---------------- constants (resident all kernel) ----------------
    cpool = ctx.enter_context(tc.tile_pool(name="consts", bufs=1))

    bcol = {}
    for p in ("q", "k"):
        t = cpool.tile([128, NT], f32, name=f"bcol_{p}")
        nc.sync.dma_start(t[:], d[f"BCOL{p}"][:])
        bcol[p] = t

    bb = {}
    for p in ("v", "o"):
        row = cpool.tile([1, H], f32, name=f"brow_{p}")
        nc.sync.dma_start(row[:], d[f"BROW{p}"][:])
        t = cpool.tile([128, H], f32, name=f"bb_{p}")
        nc.gpsimd.partition_broadcast(t[:], row[:])
        bb[p] = t

    # -------- per-batch persistent activations (q/k/v stores) --------
    bpool = ctx.enter_context(tc.tile_pool(name="acts", bufs=1))
    qT = [bpool.tile([128, S], f32r, name=f"qT{i}") for i in range(NT)]
    kT = [bpool.tile([128, S], f32r, name=f"kT{i}") for i in range(NT)]
    vs = [bpool.tile([128, VW], f32r, name=f"vs{i}") for i in range(NT)]

    psum = ctx.enter_context(tc.tile_pool(name="psum", bufs=1, space="PSUM"))

    def recon_wt(p, pool, wt):
        """wt[i][:, :] = ((LM@RM).T + W.T)[128i:128(i+1), :] in f32r.

        The rank-64 matmuls are row-packed in pairs: RM2/LMT2 hold the
        operands duplicated on partitions 0-63 and 64-127, so two
        K=64 matmuls run concurrently on disjoint PE row groups."""
        lmt = pool.tile([128, H], f32r, tag="lmt", name=f"lmt_{p}")
        nc.sync.dma_start(lmt[:], d[f"LMT{p}"][:])
        rm = pool.tile([128, H], f32r, tag="rm", name=f"rm_{p}")
        nc.sync.dma_start(rm[:], d[f"RM{p}"][:])
        for i in range(0, NT, 2):
            for oc in range(NSC):
                pw = [psum.tile([128, 512], f32, tag="proj", bufs=2,
                                name=f"pw_{p}_{i}_{oc}_{u}")
                      for u in range(2)]
                nc.tensor.matmul(
                    pw[0][:],
                    rm[0:64, i * 128:(i + 1) * 128],
                    lmt[0:64, oc * 512:(oc + 1) * 512],
                    start=True, stop=True,
                )
                nc.tensor.matmul(
                    pw[1][:],
                    rm[64:128, (i + 1) * 128:(i + 2) * 128],
                    lmt[64:128, oc * 512:(oc + 1) * 512],
                    start=True, stop=True,
                )
                for u in range(2):
                    wsrc = pool.tile([128, 512], f32, tag="wsrc", bufs=3,
                                     name=f"wsrc_{p}_{i}_{oc}_{u}")
                    nc.sync.dma_start(
                        wsrc[:], d[f"WT{p}"][(i + u) * 128:(i + u + 1) * 128,
                                             oc * 512:(oc + 1) * 512])
                    nc.vector.tensor_tensor(
                        wt[i + u][:, oc * 512:(oc + 1) * 512], pw[u][:],
                        wsrc[:], ALU.add)

    # ================= per batch =================
    for b in range(BPC):
        # ---- load xT; reconstruct + apply q/k/v projections ----
        with tc.tile_pool(name=f"xw{b}", bufs=1) as pxw:
            xt = [pxw.tile([128, S], f32r, name=f"xt{b}_{i}")
                  for i in range(NT)]
            for i in range(NT):
                nc.sync.dma_start(
                    xt[i][:], d["xT"][b, i * 128:(i + 1) * 128, :])
            wt = [pxw.tile([128, H], f32r, name=f"wt{b}_{i}")
                  for i in range(NT)]

            for p, store in (("q", qT), ("k", kT)):
                recon_wt(p, pxw, wt)
                for ot in range(NT):
                    ps = [psum.tile([128, 512], f32, tag="proj", bufs=2,
                                    name=f"ps_{p}{b}_{ot}_{i}")
                          for i in range(NSC)]
                    for it in range(NT):
                        for sc in range(NSC):
                            nc.tensor.matmul(
                                ps[sc][:],
                                wt[it][:, ot * 128:(ot + 1) * 128],
                                xt[it][:, sc * 512:(sc + 1) * 512],
                                start=(it == 0), stop=(it == NT - 1),
                            )
                    for sc in range(NSC):
                        nc.vector.tensor_scalar_add(
                            store[ot][:, sc * 512:(sc + 1) * 512],
                            ps[sc][:], bcol[p][:, ot:ot + 1])

            # ---- v in natural [s, o] layout with interleaved ones ----
            recon_wt("v", pxw, wt)
            for st in range(NT):
                grp = vs[st][:, 0:NH * (HD + 1)].rearrange(
                    "p (h d) -> p h d", d=HD + 1)
                grp_f32 = vs[st][:, 0:NH * (HD + 1)].bitcast(f32).rearrange(
                    "p (h d) -> p h d", d=HD + 1)
                nc.vector.memset(grp_f32[:, :, HD:HD + 1], 1.0)
                nc.vector.memset(vs[st][:, NH * (HD + 1):VW].bitcast(f32), 0.0)
                ps = [psum.tile([128, 512], f32, tag="proj", bufs=2,
                                name=f"ps_v{b}_{st}_{i}")
                      for i in range(NSC)]
                for it in range(NT):
                    for oc in range(NSC):
                        nc.tensor.matmul(
                            ps[oc][:],
                            xt[it][:, st * 128:(st + 1) * 128],
                            wt[it][:, oc * 512:(oc + 1) * 512],
                            start=(it == 0), stop=(it == NT - 1),
                        )
                for oc in range(NSC):
                    dst = grp[:, oc * 8:(oc + 1) * 8, 0:HD]
                    nc.vector.tensor_tensor(
                        dst, ps[oc][:],
                        bb["v"][:, oc * 512:(oc + 1) * 512], ALU.add)

        with tc.tile_pool(name=f"ct{b}", bufs=1) as pct:
            cT = [pct.tile([128, S], f32r, name=f"cT{b}_{i}")
                  for i in range(NT)]

            # ---- attention ----
            # Head pair j = heads (2j, 2j+1) live on partitions 0:64 /
            # 64:128 of qT/kT tile j. The two heads' K=64 scores
            # matmuls are emitted adjacently at tile positions (0,0)
            # and (64,0) so they run concurrently on disjoint PE row
            # groups. ctx matmuls use full M=128 lhsT windows into the
            # interleaved v store: window [65h-32, 65h+96) puts ctx_h
            # at psum partitions 32:96 and the ones-column (softmax
            # denominators) at partition 96 (h=0: window 0 with ctx at
            # 0:64, sums at 64). Junk partitions are ignored.
            with tc.tile_pool(name=f"att{b}", bufs=1) as patt:
                for j in range(NH // 2):
                    for sc in range(NSC):
                        pcs = []
                        for u in range(2):
                            pcs.append(psum.tile(
                                [128, 512], f32, tag="ctx", bufs=2,
                                name=f"pc{b}_{j}_{sc}_{u}"))
                        for kt in range(NT):
                            pss = [psum.tile([128, 512], f32, tag="att",
                                             bufs=4,
                                             name=f"pssc{b}_{j}_{sc}_{kt}_{u}")
                                   for u in range(2)]
                            nc.tensor.matmul(
                                pss[0][:],
                                kT[j][0:64, kt * 128:(kt + 1) * 128],
                                qT[j][0:64, sc * 512:(sc + 1) * 512],
                                start=True, stop=True,
                            )
                            nc.tensor.matmul(
                                pss[1][:],
                                kT[j][64:128, kt * 128:(kt + 1) * 128],
                                qT[j][64:128, sc * 512:(sc + 1) * 512],
                                start=True, stop=True,
                            )
                            for u in range(2):
                                h = 2 * j + u
                                e = patt.tile([128, 512], f32r, tag="E",
                                              bufs=8,
                                              name=f"e{b}_{j}_{sc}_{kt}_{u}")
                                nc.scalar.activation(
                                    e[:], pss[u][:], AF.Exp, scale=0.125)
                                w0 = 65 * h
                                nc.tensor.matmul(
                                    pcs[u][:],
                                    vs[kt][:, w0:w0 + 128],
                                    e[:],
                                    start=(kt == 0), stop=(kt == NT - 1),
                                )
                        for u in range(2):
                            hp = u * 64
                            srow = patt.tile([1, 512], f32, tag="srow",
                                             bufs=3,
                                             name=f"srow{b}_{j}_{sc}_{u}")
                            nc.vector.tensor_copy(srow[:], pcs[u][64:65, :])
                            recip = patt.tile([1, 512], f32, tag="recip",
                                              bufs=3,
                                              name=f"recip{b}_{j}_{sc}_{u}")
                            nc.vector.reciprocal_approx_fast(
                                recip[:], srow[:])
                            rb = patt.tile([64, 512], f32, tag="rb",
                                           bufs=3,
                                           name=f"rb{b}_{j}_{sc}_{u}")
                            nc.gpsimd.partition_broadcast(rb[:], recip[:])
                            nc.vector.tensor_tensor(
                                cT[j][hp:hp + 64, sc * 512:(sc + 1) * 512],
                                pcs[u][0:64, :],
                                rb[:], ALU.mult)

            # ---- out projection ----
            with tc.tile_pool(name=f"wo{b}", bufs=1) as pwo:
                wt = [pwo.tile([128, H], f32r, name=f"wto{b}_{i}")
                      for i in range(NT)]
                recon_wt("o", pwo, wt)
                for st in range(NT):
                    ps = [psum.tile([128, 512], f32, tag="proj", bufs=2,
                                    name=f"ps_{p}{b}_{ot}_{i}")
                          for i in range(NSC)]
                    for it in range(NT):
                        for oc in range(NSC):
                            nc.tensor.matmul(
                                ps[oc][:],
                                cT[it][:, st * 128:(st + 1) * 128],
                                wt[it][:, oc * 512:(oc + 1) * 512],
                                start=(it == 0), stop=(it == NT - 1),
                            )
                    for oc in range(NSC):
                        osb = pwo.tile([128, 512], f32, tag="osb", bufs=3, name=f"osb{b}_{st}_{oc}")
                        nc.vector.tensor_tensor(
                            osb[:], ps[oc][:],
                            bb["o"][:, oc * 512:(oc + 1) * 512], ALU.add)
                        nc.sync.dma_start(
                            d["out"][b, st * 128:(st + 1) * 128,
                                     oc * 512:(oc + 1) * 512], osb[:])


def build_nc():
    nc = bacc.Bacc("TRN2", target_bir_lowering=False, debug=False,
                   num_devices=N_CORES)
    d = {}
    d["xT"] = nc.dram_tensor("xT", [BPC, H, S], f32r,
                             kind="ExternalInput").ap()
    for p in PROJS:
        d[f"WT{p}"] = nc.dram_tensor(f"WT{p}", [H, H], f32,
                                     kind="ExternalInput").ap()
        d[f"LMT{p}"] = nc.dram_tensor(f"LMT{p}", [128, H], f32r,
                                      kind="ExternalInput").ap()
        d[f"RM{p}"] = nc.dram_tensor(f"RM{p}", [128, H], f32r,
                                     kind="ExternalInput").ap()
    for p in ("q", "k"):
        d[f"BCOL{p}"] = nc.dram_tensor(f"BCOL{p}", [128, NT], f32,
                                       kind="ExternalInput").ap()
    for p in ("v", "o"):
        d[f"BROW{p}"] = nc.dram_tensor(f"BROW{p}", [1, H], f32,
                                       kind="ExternalInput").ap()
    d["out"] = nc.dram_tensor("out", [BPC, S, H], f32,
                              kind="ExternalOutput").ap()

    with tile.TileContext(nc) as tc, ExitStack() as ctx:
        _emit(ctx, tc, d)
    nc.compile()
    return nc


_CACHE = {}


def _prep_inputs(inputs):
    """Host-side, layout-only: transposes + slicing per core."""
    g = {k: np.asarray(v, dtype=np.float32) for k, v in inputs.items()
         if k != "task"}
    shared = {}
    for p in PROJS:
        WT = np.ascontiguousarray(g["W" + p].T)
        LMT = np.ascontiguousarray(g["LM" + p].T)
        F = g["F" + p]
        if not np.all(F == 1.0):
            # fold the per-output-channel SFG scale into the transposed
            # weights (identity in practice: F is spec'd all-ones)
            WT = WT * F
            LMT = np.ascontiguousarray(LMT * F)
        shared[f"WT{p}"] = WT
        shared[f"LMT{p}"] = np.ascontiguousarray(np.vstack([LMT, LMT]))
        RM = g["RM" + p]
        shared[f"RM{p}"] = np.ascontiguousarray(np.vstack([RM, RM]))
    for p in ("q", "k"):
        shared[f"BCOL{p}"] = np.ascontiguousarray(
            (g["b" + p] * g["F" + p]).reshape(NT, 128).T)
    for p in ("v", "o"):
        shared[f"BROW{p}"] = np.ascontiguousarray(
            (g["b" + p] * g["F" + p]).reshape(1, H))
    hs = g["hidden_states"]
    in_maps = []
    for c in range(N_CORES):
        m = dict(shared)
        m["xT"] = np.ascontiguousarray(
            hs[c * BPC:(c + 1) * BPC].transpose(0, 2, 1))
        in_maps.append(m)
    return in_maps


def kernel(**inputs):
    if "nc" not in _CACHE:
        _CACHE["nc"] = build_nc()
    nc = _CACHE["nc"]
    in_maps = _prep_inputs(inputs)
    res = run_bass_kernel_spmd(nc, in_maps, list(range(N_CORES)))
    return np.concatenate([r["out"] for r in res.results], axis=0)


# revision 10
# speedup vs baseline: 1.0411x; 1.0411x over previous
"""Trainium2 Bass kernel for nn_Attention_47261820125787.

Full multi-head attention layer with low-rank-reconstructed projection
weights (w = LM @ RM + W), B=16, S=1024, H=1024, 16 heads x 64 dim.

Sharding: data-parallel over batch -- each of the 8 cores processes 2
batches with the full set of weights. No collectives.

Host-side prep is layout-only: weights are passed pre-transposed
(W.T, LM.T) and hidden_states is passed as [B, H, S] so that every DMA
is contiguous; all math (weight reconstruction, projections, attention)
runs on-device in fp32r.

On-device dataflow per core:
  wT = (LM@RM).T + W.T          reconstructed per projection in SBUF
  qT, kT = [o, s] layouts       (lhsT = wT tiles, rhs = xT tiles)
  v     = [s, o] layout         (lhsT = xT tiles, rhs = wT tiles),
          stored with a ones-column interleaved per head so the ctx
          matmul also produces softmax denominators for free
  E^T   = exp(scores^T / 8)     scores computed transposed [sk, sq];
          no max-subtraction (scores ~ N(0,1), exp can't overflow)
  ctx^T = (v|1).T @ E           [d+1, sq] per head; row 64 = sum_k E
  ctx normalized by 1/sums (vector.reciprocal + gpsimd partition
  broadcast), out = ctx @ wo.T + bo evicted in natural [s, o] layout.
"""

import numpy as np
from contextlib import ExitStack

import concourse.bass as bass
import concourse.tile as tile
from concourse import bacc, mybir
from concourse.bass_utils import run_bass_kernel_spmd

B, S, H, NH, HD = 16, 1024, 1024, 16, 64
KR = 64             # low-rank dim
N_CORES = 8
BPC = B // N_CORES  # batches per core

f32 = mybir.dt.float32
f32r = mybir.dt.float32r
AF = mybir.ActivationFunctionType
ALU = mybir.AluOpType

PROJS = ("q", "k", "v", "o")
NT = H // 128       # 8 partition tiles over hidden dim
NSC = S // 512      # 2 free chunks over sequence
VW = NH * (HD + 1) + 64  # 64 cols + ones col per head + window pad


def _emit(ctx: ExitStack, tc: tile.TileContext, d: dict):
    nc = tc.nc

    # ---------------- constants (resident all kernel) ----------------
    cpool = ctx.enter_context(tc.tile_pool(name="consts", bufs=1))

    bcol = {}
    for p in ("q", "k"):
        t = cpool.tile([128, NT], f32, name=f"bcol_{p}")
        nc.sync.dma_start(t[:], d[f"BCOL{p}"][:])
        bcol[p] = t

    bb = {}
    for p in ("v", "o"):
        row = cpool.tile([1, H], f32, name=f"brow_{p}")
        nc.sync.dma_start(row[:], d[f"BROW{p}"][:])
        t = cpool.tile([128, H], f32, name=f"bb_{p}")
        nc.gpsimd.partition_broadcast(t[:], row[:])
        bb[p] = t

    # -------- per-batch persistent activations (q/k/v stores) --------
    bpool = ctx.enter_context(tc.tile_pool(name="acts", bufs=1))
    qT = [bpool.tile([128, S], f32r, name=f"qT{i}") for i in range(NT)]
    kT = [bpool.tile([128, S], f32r, name=f"kT{i}") for i in range(NT)]
    vs = [bpool.tile([128, VW], f32r, name=f"vs{i}") for i in range(NT)]

    psum = ctx.enter_context(tc.tile_pool(name="psum", bufs=1, space="PSUM"))

    def recon_wt(p, pool, wt):
        """wt[i][:, :] = ((LM@RM).T + W.T)[128i:128(i+1), :] in f32r.

        The rank-64 matmuls are row-packed in pairs: RM2/LMT2 hold the
        operands duplicated on partitions 0-63 and 64-127, so two
        K=64 matmuls run concurrently on disjoint PE row groups."""
        lmt = pool.tile([128, H], f32r, tag="lmt", name=f"lmt_{p}")
        nc.sync.dma_start(lmt[:], d[f"LMT{p}"][:])
        rm = pool.tile([128, H], f32r, tag="rm", name=f"rm_{p}")
        nc.sync.dma_start(rm[:], d[f"RM{p}"][:])
        for i in range(0, NT, 2):
            for oc in range(NSC):
                pw = [psum.tile([128, 512], f32, tag="proj", bufs=2,
                                name=f"pw_{p}_{i}_{oc}_{u}")
                      for u in range(2)]
                nc.tensor.matmul(
                    pw[0][:],
                    rm[0:64, i * 128:(i + 1) * 128],
                    lmt[0:64, oc * 512:(oc + 1) * 512],
                    start=True, stop=True,
                )
                nc.tensor.matmul(
                    pw[1][:],
                    rm[64:128, (i + 1) * 128:(i + 2) * 128],
                    lmt[64:128, oc * 512:(oc + 1) * 512],
                    start=True, stop=True,
                )
                for u in range(2):
                    wsrc = pool.tile([128, 512], f32, tag="wsrc", bufs=6,
                                     name=f"wsrc_{p}_{i}_{oc}_{u}")
                    nc.sync.dma_start(
                        wsrc[:], d[f"WT{p}"][(i + u) * 128:(i + u + 1) * 128,
                                             oc * 512:(oc + 1) * 512])
                    nc.vector.tensor_tensor(
                        wt[i + u][:, oc * 512:(oc + 1) * 512], pw[u][:],
                        wsrc[:], ALU.add)

    # ================= per batch =================
    for b in range(BPC):
        # ---- load xT; reconstruct + apply q/k/v projections ----
        with tc.tile_pool(name=f"xw{b}", bufs=1) as pxw:
            xt = [pxw.tile([128, S], f32r, name=f"xt{b}_{i}")
                  for i in range(NT)]
            for i in range(NT):
                nc.sync.dma_start(
                    xt[i][:], d["xT"][b, i * 128:(i + 1) * 128, :])
            wt = [pxw.tile([128, H], f32r, name=f"wt{b}_{i}")
                  for i in range(NT)]

            for p, store in (("q", qT), ("k", kT)):
                recon_wt(p, pxw, wt)
                for ot in range(NT):
                    ps = [psum.tile([128, 512], f32, tag="proj", bufs=2,
                                    name=f"ps_{p}{b}_{ot}_{i}")
                          for i in range(NSC)]
                    for it in range(NT):
                        for sc in range(NSC):
                            nc.tensor.matmul(
                                ps[sc][:],
                                wt[it][:, ot * 128:(ot + 1) * 128],
                                xt[it][:, sc * 512:(sc + 1) * 512],
                                start=(it == 0), stop=(it == NT - 1),
                            )
                    for sc in range(NSC):
                        nc.vector.tensor_scalar_add(
                            store[ot][:, sc * 512:(sc + 1) * 512],
                            ps[sc][:], bcol[p][:, ot:ot + 1])

            # ---- v in natural [s, o] layout with interleaved ones ----
            recon_wt("v", pxw, wt)
            for st in range(NT):
                grp = vs[st][:, 0:NH * (HD + 1)].rearrange(
                    "p (h d) -> p h d", d=HD + 1)
                grp_f32 = vs[st][:, 0:NH * (HD + 1)].bitcast(f32).rearrange(
                    "p (h d) -> p h d", d=HD + 1)
                nc.vector.memset(grp_f32[:, :, HD:HD + 1], 1.0)
                nc.vector.memset(vs[st][:, NH * (HD + 1):VW].bitcast(f32), 0.0)
                ps = [psum.tile([128, 512], f32, tag="proj", bufs=2,
                                name=f"ps_v{b}_{st}_{i}")
                      for i in range(NSC)]
                for it in range(NT):
                    for oc in range(NSC):
                        nc.tensor.matmul(
                            ps[oc][:],
                            xt[it][:, st * 128:(st + 1) * 128],
                            wt[it][:, oc * 512:(oc + 1) * 512],
                            start=(it == 0), stop=(it == NT - 1),
                        )
                for oc in range(NSC):
                    dst = grp[:, oc * 8:(oc + 1) * 8, 0:HD]
                    nc.vector.tensor_tensor(
                        dst, ps[oc][:],
                        bb["v"][:, oc * 512:(oc + 1) * 512], ALU.add)

        with tc.tile_pool(name=f"ct{b}", bufs=1) as pct:
            cT = [pct.tile([128, S], f32r, name=f"cT{b}_{i}")
                  for i in range(NT)]

            # ---- attention ----
            # Head pair j = heads (2j, 2j+1) live on partitions 0:64 /
            # 64:128 of qT/kT tile j. The two heads' K=64 scores
            # matmuls are emitted adjacently at tile positions (0,0)
            # and (64,0) so they run concurrently on disjoint PE row
            # groups. ctx matmuls use full M=128 lhsT windows into the
            # interleaved v store: window [65h-32, 65h+96) puts ctx_h
            # at psum partitions 32:96 and the ones-column (softmax
            # denominators) at partition 96 (h=0: window 0 with ctx at
            # 0:64, sums at 64). Junk partitions are ignored.
            with tc.tile_pool(name=f"att{b}", bufs=1) as patt:
                for j in range(NH // 2):
                    for sc in range(NSC):
                        pcs = [psum.tile([128, 512], f32, tag="ctx", bufs=2,
                                         name=f"pc{b}_{j}_{sc}_{u}")
                               for u in range(2)]
                        es = {}

                        def scores_step(kt):
                            pss = [psum.tile([128, 512], f32, tag="att",
                                             bufs=4,
                                             name=f"pssc{b}_{j}_{sc}_{kt}_{u}")
                                   for u in range(2)]
                            nc.tensor.matmul(
                                pss[0][:],
                                kT[j][0:64, kt * 128:(kt + 1) * 128],
                                qT[j][0:64, sc * 512:(sc + 1) * 512],
                                start=True, stop=True,
                            )
                            nc.tensor.matmul(
                                pss[1][:],
                                kT[j][64:128, kt * 128:(kt + 1) * 128],
                                qT[j][64:128, sc * 512:(sc + 1) * 512],
                                start=True, stop=True,
                            )
                            for u in range(2):
                                e = patt.tile([128, 512], f32r, tag="E",
                                              bufs=10,
                                              name=f"e{b}_{j}_{sc}_{kt}_{u}")
                                nc.scalar.activation(
                                    e[:], pss[u][:], AF.Exp, scale=0.125)
                                es[kt, u] = e

                        def ctx_step(kt):
                            for u in range(2):
                                h = 2 * j + u
                                nc.tensor.matmul(
                                    pcs[u][:],
                                    vs[kt][:, 65 * h:65 * h + 128],
                                    es[kt, u][:],
                                    start=(kt == 0), stop=(kt == NT - 1),
                                )

                        # software pipeline: ctx MMs trail scores by one
                        # kt step so the exp (ACT) latency is hidden
                        scores_step(0)
                        for kt in range(1, NT):
                            scores_step(kt)
                            ctx_step(kt - 1)
                        ctx_step(NT - 1)

                        for u in range(2):
                            hp = u * 64
                            srow = patt.tile([1, 512], f32, tag="srow",
                                             bufs=3,
                                             name=f"srow{b}_{j}_{sc}_{u}")
                            nc.vector.tensor_copy(srow[:], pcs[u][64:65, :])
                            recip = patt.tile([1, 512], f32, tag="recip",
                                              bufs=3,
                                              name=f"recip{b}_{j}_{sc}_{u}")
                            nc.vector.reciprocal_approx_fast(
                                recip[:], srow[:])
                            rb = patt.tile([64, 512], f32, tag="rb",
                                           bufs=3,
                                           name=f"rb{b}_{j}_{sc}_{u}")
                            nc.gpsimd.partition_broadcast(rb[:], recip[:])
                            nc.vector.tensor_tensor(
                                cT[j][hp:hp + 64, sc * 512:(sc + 1) * 512],
                                pcs[u][0:64, :],
                                rb[:], ALU.mult)

            # ---- out projection ----
            with tc.tile_pool(name=f"wo{b}", bufs=1) as pwo:
                wt = [pwo.tile([128, H], f32r, name=f"wto{b}_{i}")
                      for i in range(NT)]
                recon_wt("o", pwo, wt)
                for st in range(NT):
                    ps = [psum.tile([128, 512], f32, tag="proj", bufs=2,
                                    name=f"ps_{p}{b}_{ot}_{i}")
                          for i in range(NSC)]
                    for it in range(NT):
                        for oc in range(NSC):
                            nc.tensor.matmul(
                                ps[oc][:],
                                cT[it][:, st * 128:(st + 1) * 128],
                                wt[it][:, oc * 512:(oc + 1) * 512],
                                start=(it == 0), stop=(it == NT - 1),
                            )
                    for oc in range(NSC):
                        osb = pwo.tile([128, 512], f32, tag="osb", bufs=3, name=f"osb{b}_{st}_{oc}")
                        nc.vector.tensor_tensor(
                            osb[:], ps[oc][:],
                            bb["o"][:, oc * 512:(oc + 1) * 512], ALU.add)
                        nc.sync.dma_start(
                            d["out"][b, st * 128:(st + 1) * 128,
                                     oc * 512:(oc + 1) * 512], osb[:])


def build_nc():
    nc = bacc.Bacc("TRN2", target_bir_lowering=False, debug=False,
                   num_devices=N_CORES)
    d = {}
    d["xT"] = nc.dram_tensor("xT", [BPC, H, S], f32r,
                             kind="ExternalInput").ap()
    for p in PROJS:
        d[f"WT{p}"] = nc.dram_tensor(f"WT{p}", [H, H], f32,
                                     kind="ExternalInput").ap()
        d[f"LMT{p}"] = nc.dram_tensor(f"LMT{p}", [128, H], f32r,
                                      kind="ExternalInput").ap()
        d[f"RM{p}"] = nc.dram_tensor(f"RM{p}", [128, H], f32r,
                                     kind="ExternalInput").ap()
    for p in ("q", "k"):
        d[f"BCOL{p}"] = nc.dram_tensor(f"BCOL{p}", [128, NT], f32,
                                       kind="ExternalInput").ap()
    for p in ("v", "o"):
        d[f"BROW{p}"] = nc.dram_tensor(f"BROW{p}", [1, H], f32,
                                       kind="ExternalInput").ap()
    d["out"] = nc.dram_tensor("out", [BPC, S, H], f32,
                              kind="ExternalOutput").ap()

    with tile.TileContext(nc) as tc, ExitStack() as ctx:
        _emit(ctx, tc, d)
    nc.compile()
    return nc


_CACHE = {}


def _prep_inputs(inputs):
    """Host-side, layout-only: transposes + slicing per core."""
    g = {k: np.asarray(v, dtype=np.float32) for k, v in inputs.items()
         if k != "task"}
    shared = {}
    for p in PROJS:
        WT = np.ascontiguousarray(g["W" + p].T)
        LMT = np.ascontiguousarray(g["LM" + p].T)
        F = g["F" + p]
        if not np.all(F == 1.0):
            # fold the per-output-channel SFG scale into the transposed
            # weights (identity in practice: F is spec'd all-ones)
            WT = WT * F
            LMT = np.ascontiguousarray(LMT * F)
        shared[f"WT{p}"] = WT
        shared[f"LMT{p}"] = np.ascontiguousarray(np.vstack([LMT, LMT]))
        RM = g["RM" + p]
        shared[f"RM{p}"] = np.ascontiguousarray(np.vstack([RM, RM]))
    for p in ("q", "k"):
        shared[f"BCOL{p}"] = np.ascontiguousarray(
            (g["b" + p] * g["F" + p]).reshape(NT, 128).T)
    for p in ("v", "o"):
        shared[f"BROW{p}"] = np.ascontiguousarray(
            (g["b" + p] * g["F" + p]).reshape(1, H))
    hs = g["hidden_states"]
    in_maps = []
    for c in range(N_CORES):
        m = dict(shared)
        m["xT"] = np.ascontiguousarray(
            hs[c * BPC:(c + 1) * BPC].transpose(0, 2, 1))
        in_maps.append(m)
    return in_maps


def kernel(**inputs):
    if "nc" not in _CACHE:
        _CACHE["nc"] = build_nc()
    nc = _CACHE["nc"]
    in_maps = _prep_inputs(inputs)
    res = run_bass_kernel_spmd(nc, in_maps, list(range(N_CORES)))
    return np.concatenate([r["out"] for r in res.results], axis=0)


# revision 17
# speedup vs baseline: 1.3300x; 1.2774x over previous
"""Trainium2 Bass kernel for nn_Attention_47261820125787.

Full multi-head attention layer with low-rank-reconstructed projection
weights (w = LM @ RM + W), B=16, S=1024, H=1024, 16 heads x 64 dim.

Sharding: data-parallel over batch -- each of the 8 cores processes 2
batches with the full set of weights. No collectives.

Host-side prep is layout-only + fp16 casts: weights are passed
pre-transposed (W.T, LM.T duplicated across both partition halves for
row-packed rank-64 matmuls) and hidden_states as [B, H, S] so every
DMA is contiguous.

On-device dataflow per core (fp16 operands, fp32 PSUM accumulation):
  wT = (LM@RM).T + W.T         reconstructed per projection in SBUF
  qT, kT = [o, s] layouts      (lhsT = wT tiles, rhs = xT tiles)
  v      = [s, o] layout       (lhsT = xT tiles, rhs = wT tiles),
           stored with a ones-column interleaved per head so the ctx
           matmul also produces softmax denominators for free
  E^T    = exp(scores^T / 8)   (bf16: raw scores reach ~103, exp needs
           f32-class range) scores computed transposed [sk, sq]; the
           two heads of a pair run as row-packed K=64 matmul pairs on
           disjoint PE row groups (concurrent). No max-subtraction:
           scores ~ N(0,1), exp cannot overflow.
  ctx^T  = (v|1).T @ E         full M=128 lhsT windows [65h, 65h+128)
           put ctx_h at psum partitions 0:64, sums at 64, junk above.
  ctx normalized by 1/sums (fast reciprocal + gpsimd partition
  broadcast), out = ctx @ wo.T + bo evicted in natural [s, o] layout.

Both batches' activation stores are resident simultaneously (fp16
halves the footprint) so batch 1's projections overlap batch 0's
ACT-bound attention on the PE.
"""

import numpy as np
from contextlib import ExitStack

import concourse.bass as bass
import concourse.tile as tile
from concourse import bacc, mybir
from concourse.bass_utils import run_bass_kernel_spmd

B, S, H, NH, HD = 16, 1024, 1024, 16, 64
KR = 64             # low-rank dim
N_CORES = 8
BPC = B // N_CORES  # batches per core

f32 = mybir.dt.float32
fp16 = mybir.dt.float16
bf16 = mybir.dt.bfloat16
AF = mybir.ActivationFunctionType
ALU = mybir.AluOpType

PROJS = ("q", "k", "v", "o")
NT = H // 128       # 8 partition tiles over hidden dim
NSC = S // 512      # 2 free chunks over sequence
VW = NH * (HD + 1) + 64  # 64 cols + ones col per head + window pad


def _emit(ctx: ExitStack, tc: tile.TileContext, d: dict):
    nc = tc.nc

    # ---------------- constants (resident all kernel) ----------------
    cpool = ctx.enter_context(tc.tile_pool(name="consts", bufs=1))

    bcol = {}
    for p in ("q", "k"):
        t = cpool.tile([128, NT], f32, name=f"bcol_{p}")
        nc.sync.dma_start(t[:], d[f"BCOL{p}"][:])
        bcol[p] = t

    bb = {}
    for p in ("v", "o"):
        row = cpool.tile([1, H], f32, name=f"brow_{p}")
        nc.sync.dma_start(row[:], d[f"BROW{p}"][:])
        t = cpool.tile([128, H], f32, name=f"bb_{p}")
        nc.gpsimd.partition_broadcast(t[:], row[:])
        bb[p] = t

    # ---- per-batch activation stores: both batches resident (fp16) ----
    bpool = ctx.enter_context(tc.tile_pool(name="acts", bufs=1))
    qT, kT, vs, cT = {}, {}, {}, {}
    for b in range(BPC):
        qT[b] = [bpool.tile([128, S], fp16, name=f"qT{b}_{i}")
                 for i in range(NT)]
        kT[b] = [bpool.tile([128, S], fp16, name=f"kT{b}_{i}")
                 for i in range(NT)]
        vs[b] = [bpool.tile([128, VW], bf16, name=f"vs{b}_{i}")
                 for i in range(NT)]
        cT[b] = [bpool.tile([128, S], fp16, name=f"cT{b}_{i}")
                 for i in range(NT)]

    epool = ctx.enter_context(tc.tile_pool(name="epool", bufs=1))
    spool = ctx.enter_context(tc.tile_pool(name="small", bufs=1))
    wpool = ctx.enter_context(tc.tile_pool(name="wts", bufs=1))
    opool = ctx.enter_context(tc.tile_pool(name="outs", bufs=1))
    psum = ctx.enter_context(tc.tile_pool(name="psum", bufs=1, space="PSUM"))

    def recon_wt(p, wt, tag):
        """wt[i][:, :] = ((LM@RM).T + W.T)[128i:128(i+1), :] in fp16.

        Rank-64 matmuls row-packed in pairs (operands duplicated on
        both partition halves host-side)."""
        lmt = wpool.tile([128, H], fp16, tag="lmt", name=f"lmt_{p}_{tag}")
        nc.sync.dma_start(lmt[:], d[f"LMT{p}"][:])
        rm = wpool.tile([128, H], fp16, tag="rm", name=f"rm_{p}_{tag}")
        nc.sync.dma_start(rm[:], d[f"RM{p}"][:])
        for i in range(0, NT, 2):
            for oc in range(NSC):
                pw = [psum.tile([128, 512], f32, tag=tag, bufs=2,
                                name=f"pw_{p}{tag}_{i}_{oc}_{u}")
                      for u in range(2)]
                nc.tensor.matmul(
                    pw[0][:],
                    rm[0:64, i * 128:(i + 1) * 128],
                    lmt[0:64, oc * 512:(oc + 1) * 512],
                    start=True, stop=True,
                )
                nc.tensor.matmul(
                    pw[1][:],
                    rm[64:128, (i + 1) * 128:(i + 2) * 128],
                    lmt[64:128, oc * 512:(oc + 1) * 512],
                    start=True, stop=True,
                )
                for u in range(2):
                    wsrc = wpool.tile([128, 512], fp16, tag="wsrc", bufs=2,
                                      name=f"wsrc_{p}{tag}_{i}_{oc}_{u}")
                    nc.sync.dma_start(
                        wsrc[:], d[f"WT{p}"][(i + u) * 128:(i + u + 1) * 128,
                                             oc * 512:(oc + 1) * 512])
                    nc.vector.tensor_tensor(
                        wt[i + u][:, oc * 512:(oc + 1) * 512], pw[u][:],
                        wsrc[:], ALU.add)

    # ================= phases =================
    def load_x_and_project(b, xt, wt):
        for i in range(NT):
            nc.sync.dma_start(
                xt[i][:], d["xT"][b, i * 128:(i + 1) * 128, :])
        for p, store in (("q", qT[b]), ("k", kT[b])):
            recon_wt(p, wt, "proj")
            for ot in range(NT):
                ps = [psum.tile([128, 512], f32, tag="proj", bufs=2,
                                name=f"ps_{p}{b}_{ot}_{i}")
                      for i in range(NSC)]
                for it in range(NT):
                    for sc in range(NSC):
                        nc.tensor.matmul(
                            ps[sc][:],
                            wt[it][:, ot * 128:(ot + 1) * 128],
                            xt[it][:, sc * 512:(sc + 1) * 512],
                            start=(it == 0), stop=(it == NT - 1),
                        )
                for sc in range(NSC):
                    nc.vector.tensor_scalar_add(
                        store[ot][:, sc * 512:(sc + 1) * 512],
                        ps[sc][:], bcol[p][:, ot:ot + 1])

        # ---- v in natural [s, o] layout with interleaved ones ----
        recon_wt("v", wt, "proj")
        for st in range(NT):
            grp = vs[b][st][:, 0:NH * (HD + 1)].rearrange(
                "p (h d) -> p h d", d=HD + 1)
            nc.vector.memset(grp[:, :, HD:HD + 1], 1.0)
            nc.vector.memset(vs[b][st][:, NH * (HD + 1):VW], 0.0)
            ps = [psum.tile([128, 512], f32, tag="proj", bufs=2,
                            name=f"ps_v{b}_{st}_{i}")
                  for i in range(NSC)]
            for it in range(NT):
                for oc in range(NSC):
                    nc.tensor.matmul(
                        ps[oc][:],
                        xt[it][:, st * 128:(st + 1) * 128],
                        wt[it][:, oc * 512:(oc + 1) * 512],
                        start=(it == 0), stop=(it == NT - 1),
                    )
            for oc in range(NSC):
                dst = grp[:, oc * 8:(oc + 1) * 8, 0:HD]
                nc.vector.tensor_tensor(
                    dst, ps[oc][:],
                    bb["v"][:, oc * 512:(oc + 1) * 512], ALU.add)

    def attention(b):
        for j in range(NH // 2):
            for sc in range(NSC):
                pcs = [psum.tile([128, 512], f32, tag="ctx", bufs=2,
                                 name=f"pc{b}_{j}_{sc}_{u}")
                       for u in range(2)]
                es = {}

                def scores_step(kt):
                    pss = [psum.tile([128, 512], f32, tag="att", bufs=2,
                                     name=f"pssc{b}_{j}_{sc}_{kt}_{u}")
                           for u in range(2)]
                    nc.tensor.matmul(
                        pss[0][:],
                        kT[b][j][0:64, kt * 128:(kt + 1) * 128],
                        qT[b][j][0:64, sc * 512:(sc + 1) * 512],
                        start=True, stop=True,
                    )
                    nc.tensor.matmul(
                        pss[1][:],
                        kT[b][j][64:128, kt * 128:(kt + 1) * 128],
                        qT[b][j][64:128, sc * 512:(sc + 1) * 512],
                        start=True, stop=True,
                    )
                    for u in range(2):
                        e = epool.tile([128, 512], bf16, tag="E", bufs=4,
                                       name=f"e{b}_{j}_{sc}_{kt}_{u}")
                        nc.scalar.activation(
                            e[:], pss[u][:], AF.Exp, scale=0.125)
                        es[kt, u] = e

                def ctx_step(kt):
                    for u in range(2):
                        h = 2 * j + u
                        nc.tensor.matmul(
                            pcs[u][:],
                            vs[b][kt][:, 65 * h:65 * h + 128],
                            es[kt, u][:],
                            start=(kt == 0), stop=(kt == NT - 1),
                        )

                # software pipeline: ctx matmuls trail scores by one kt
                scores_step(0)
                for kt in range(1, NT):
                    scores_step(kt)
                    ctx_step(kt - 1)
                ctx_step(NT - 1)

                for u in range(2):
                    hp = u * 64
                    srow = spool.tile([1, 512], f32, tag="srow", bufs=1,
                                      name=f"srow{b}_{j}_{sc}_{u}")
                    nc.vector.tensor_copy(srow[:], pcs[u][64:65, :])
                    recip = spool.tile([1, 512], f32, tag="recip", bufs=1,
                                       name=f"recip{b}_{j}_{sc}_{u}")
                    nc.vector.reciprocal_approx_fast(recip[:], srow[:])
                    rb = spool.tile([64, 512], f32, tag="rb", bufs=1,
                                    name=f"rb{b}_{j}_{sc}_{u}")
                    nc.gpsimd.partition_broadcast(rb[:], recip[:])
                    nc.vector.tensor_tensor(
                        cT[b][j][hp:hp + 64, sc * 512:(sc + 1) * 512],
                        pcs[u][0:64, :], rb[:], ALU.mult)

    def out_projection(b, wt):
        for st in range(NT):
            ps = [psum.tile([128, 512], f32, tag="oproj", bufs=2,
                            name=f"ps_o{b}_{st}_{i}")
                  for i in range(NSC)]
            for it in range(NT):
                for oc in range(NSC):
                    nc.tensor.matmul(
                        ps[oc][:],
                        cT[b][it][:, st * 128:(st + 1) * 128],
                        wt[it][:, oc * 512:(oc + 1) * 512],
                        start=(it == 0), stop=(it == NT - 1),
                    )
            for oc in range(NSC):
                osb = opool.tile([128, 512], f32, tag="osb", bufs=2,
                                 name=f"osb{b}_{st}_{oc}")
                nc.vector.tensor_tensor(
                    osb[:], ps[oc][:],
                    bb["o"][:, oc * 512:(oc + 1) * 512], ALU.add)
                nc.sync.dma_start(
                    d["out"][b, st * 128:(st + 1) * 128,
                             oc * 512:(oc + 1) * 512], osb[:])

    # ================= schedule =================
    # batch 0 projections; batch 0 attention; batch 1 projections are
    # emitted before batch 0's out-projection so the PE has projection
    # work while batch 0's ACT-bound attention runs. wt_o is
    # reconstructed once into batch 1's wt tiles (after its v
    # projection) and serves both out-projections.
    with tc.tile_pool(name="xw0", bufs=1) as pxw0:
        xt0 = [pxw0.tile([128, S], fp16, name=f"xt0_{i}")
               for i in range(NT)]
        wt0 = [pxw0.tile([128, H], fp16, name=f"wt0_{i}")
               for i in range(NT)]
        load_x_and_project(0, xt0, wt0)
    attention(0)
    with tc.tile_pool(name="xw1", bufs=1) as pxw1:
        xt1 = [pxw1.tile([128, S], fp16, name=f"xt1_{i}")
               for i in range(NT)]
        wt1 = [pxw1.tile([128, H], fp16, name=f"wt1_{i}")
               for i in range(NT)]
        load_x_and_project(1, xt1, wt1)
        recon_wt("o", wt1, "oproj")
        out_projection(0, wt1)
        attention(1)
        out_projection(1, wt1)


def build_nc():
    nc = bacc.Bacc("TRN2", target_bir_lowering=False, debug=False,
                   num_devices=N_CORES)
    d = {}
    d["xT"] = nc.dram_tensor("xT", [BPC, H, S], fp16,
                             kind="ExternalInput").ap()
    for p in PROJS:
        d[f"WT{p}"] = nc.dram_tensor(f"WT{p}", [H, H], fp16,
                                     kind="ExternalInput").ap()
        d[f"LMT{p}"] = nc.dram_tensor(f"LMT{p}", [128, H], fp16,
                                      kind="ExternalInput").ap()
        d[f"RM{p}"] = nc.dram_tensor(f"RM{p}", [128, H], fp16,
                                     kind="ExternalInput").ap()
    for p in ("q", "k"):
        d[f"BCOL{p}"] = nc.dram_tensor(f"BCOL{p}", [128, NT], f32,
                                       kind="ExternalInput").ap()
    for p in ("v", "o"):
        d[f"BROW{p}"] = nc.dram_tensor(f"BROW{p}", [1, H], f32,
                                       kind="ExternalInput").ap()
    d["out"] = nc.dram_tensor("out", [BPC, S, H], f32,
                              kind="ExternalOutput").ap()

    with tile.TileContext(nc) as tc, ExitStack() as ctx:
        _emit(ctx, tc, d)
    nc.compile()
    return nc


_CACHE = {}


def _prep_inputs(inputs):
    """Host-side prep: transposes + slicing per core + fp16 casts."""
    g = {k: np.asarray(v, dtype=np.float32) for k, v in inputs.items()
         if k != "task"}
    shared = {}
    for p in PROJS:
        WT = np.ascontiguousarray(g["W" + p].T)
        LMT = np.ascontiguousarray(g["LM" + p].T)
        F = g["F" + p]
        if not np.all(F == 1.0):
            # fold the per-output-channel SFG scale into the transposed
            # weights (identity in practice: F is spec'd all-ones)
            WT = WT * F
            LMT = np.ascontiguousarray(LMT * F)
        shared[f"WT{p}"] = WT.astype(np.float16)
        LMT16 = LMT.astype(np.float16)
        RM16 = g["RM" + p].astype(np.float16)
        shared[f"LMT{p}"] = np.ascontiguousarray(np.vstack([LMT16, LMT16]))
        shared[f"RM{p}"] = np.ascontiguousarray(np.vstack([RM16, RM16]))
    for p in ("q", "k"):
        shared[f"BCOL{p}"] = np.ascontiguousarray(
            (g["b" + p] * g["F" + p]).reshape(NT, 128).T)
    for p in ("v", "o"):
        shared[f"BROW{p}"] = np.ascontiguousarray(
            (g["b" + p] * g["F" + p]).reshape(1, H))
    hs = g["hidden_states"]
    in_maps = []
    for c in range(N_CORES):
        m = dict(shared)
        m["xT"] = np.ascontiguousarray(
            hs[c * BPC:(c + 1) * BPC].transpose(0, 2, 1)).astype(np.float16)
        in_maps.append(m)
    return in_maps


def kernel(**inputs):
    if "nc" not in _CACHE:
        _CACHE["nc"] = build_nc()
    nc = _CACHE["nc"]
    in_maps = _prep_inputs(inputs)
    res = run_bass_kernel_spmd(nc, in_maps, list(range(N_CORES)))
    return np.concatenate([r["out"] for r in res.results], axis=0)


# revision 18
# speedup vs baseline: 1.4408x; 1.0834x over previous
"""Trainium2 Bass kernel for nn_Attention_47261820125787.

Full multi-head attention layer with low-rank-reconstructed projection
weights (w = LM @ RM + W), B=16, S=1024, H=1024, 16 heads x 64 dim.

Sharding: data-parallel over batch -- each of the 8 cores processes 2
batches with the full set of weights. No collectives.

Host-side prep is layout-only + fp16 casts: weights are passed
pre-transposed (W.T, LM.T duplicated across both partition halves for
row-packed rank-64 matmuls) and hidden_states as [B, H, S] so every
DMA is contiguous.

On-device dataflow per core (fp16 operands, fp32 PSUM accumulation):
  wT = (LM@RM).T + W.T         reconstructed per projection in SBUF
  qT, kT = [o, s] layouts      (lhsT = wT tiles, rhs = xT tiles)
  v      = [s, o] layout       (lhsT = xT tiles, rhs = wT tiles),
           stored with a ones-column interleaved per head so the ctx
           matmul also produces softmax denominators for free
  E^T    = exp(scores^T / 8)   (bf16: raw scores reach ~103, exp needs
           f32-class range) scores computed transposed [sk, sq]; the
           two heads of a pair run as row-packed K=64 matmul pairs on
           disjoint PE row groups (concurrent). No max-subtraction:
           scores ~ N(0,1), exp cannot overflow.
  ctx^T  = (v|1).T @ E         full M=128 lhsT windows [65h, 65h+128)
           put ctx_h at psum partitions 0:64, sums at 64, junk above.
  ctx normalized by 1/sums (fast reciprocal + gpsimd partition
  broadcast), out = ctx @ wo.T + bo evicted in natural [s, o] layout.

Both batches' activation stores are resident simultaneously (fp16
halves the footprint) so batch 1's projections overlap batch 0's
ACT-bound attention on the PE.
"""

import numpy as np
from contextlib import ExitStack

import concourse.bass as bass
import concourse.tile as tile
from concourse import bacc, mybir
from concourse.bass_utils import run_bass_kernel_spmd

B, S, H, NH, HD = 16, 1024, 1024, 16, 64
KR = 64             # low-rank dim
N_CORES = 8
BPC = B // N_CORES  # batches per core

f32 = mybir.dt.float32
fp16 = mybir.dt.float16
bf16 = mybir.dt.bfloat16
AF = mybir.ActivationFunctionType
ALU = mybir.AluOpType

PROJS = ("q", "k", "v", "o")
NT = H // 128       # 8 partition tiles over hidden dim
NSC = S // 512      # 2 free chunks over sequence
VW = NH * (HD + 1) + 64  # 64 cols + ones col per head + window pad


def _emit(ctx: ExitStack, tc: tile.TileContext, d: dict):
    nc = tc.nc

    # ---------------- constants (resident all kernel) ----------------
    cpool = ctx.enter_context(tc.tile_pool(name="consts", bufs=1))

    bcol = {}
    for p in ("q", "k"):
        t = cpool.tile([128, NT], f32, name=f"bcol_{p}")
        nc.sync.dma_start(t[:], d[f"BCOL{p}"][:])
        bcol[p] = t

    bb = {}
    for p in ("v", "o"):
        row = cpool.tile([1, H], f32, name=f"brow_{p}")
        nc.sync.dma_start(row[:], d[f"BROW{p}"][:])
        t = cpool.tile([128, H], f32, name=f"bb_{p}")
        nc.gpsimd.partition_broadcast(t[:], row[:])
        bb[p] = t

    # ---- per-batch activation stores: both batches resident (fp16) ----
    bpool = ctx.enter_context(tc.tile_pool(name="acts", bufs=1))
    qT, kT, vs, cT = {}, {}, {}, {}
    for b in range(BPC):
        qT[b] = [bpool.tile([128, S], fp16, name=f"qT{b}_{i}")
                 for i in range(NT)]
        kT[b] = [bpool.tile([128, S], fp16, name=f"kT{b}_{i}")
                 for i in range(NT)]
        vs[b] = [bpool.tile([128, VW], bf16, name=f"vs{b}_{i}")
                 for i in range(NT)]
        cT[b] = [bpool.tile([128, S], fp16, name=f"cT{b}_{i}")
                 for i in range(NT)]

    epool = ctx.enter_context(tc.tile_pool(name="epool", bufs=1))
    spool = ctx.enter_context(tc.tile_pool(name="small", bufs=1))
    wpool = ctx.enter_context(tc.tile_pool(name="wts", bufs=1))
    opool = ctx.enter_context(tc.tile_pool(name="outs", bufs=1))
    psum = ctx.enter_context(tc.tile_pool(name="psum", bufs=1, space="PSUM"))

    def recon_wt(p, wt, tag):
        """wt[i][:, :] = ((LM@RM).T + W.T)[128i:128(i+1), :] in fp16.

        Rank-64 matmuls row-packed in pairs (operands duplicated on
        both partition halves host-side)."""
        lmt = wpool.tile([128, H], fp16, tag="lmt", name=f"lmt_{p}_{tag}")
        nc.sync.dma_start(lmt[:], d[f"LMT{p}"][:])
        rm = wpool.tile([128, H], fp16, tag="rm", name=f"rm_{p}_{tag}")
        nc.sync.dma_start(rm[:], d[f"RM{p}"][:])
        for i in range(0, NT, 2):
            for oc in range(NSC):
                pw = [psum.tile([128, 512], f32, tag=tag, bufs=2,
                                name=f"pw_{p}{tag}_{i}_{oc}_{u}")
                      for u in range(2)]
                nc.tensor.matmul(
                    pw[0][:],
                    rm[0:64, i * 128:(i + 1) * 128],
                    lmt[0:64, oc * 512:(oc + 1) * 512],
                    start=True, stop=True,
                )
                nc.tensor.matmul(
                    pw[1][:],
                    rm[64:128, (i + 1) * 128:(i + 2) * 128],
                    lmt[64:128, oc * 512:(oc + 1) * 512],
                    start=True, stop=True,
                )
                for u in range(2):
                    wsrc = wpool.tile([128, 512], fp16, tag="wsrc", bufs=2,
                                      name=f"wsrc_{p}{tag}_{i}_{oc}_{u}")
                    nc.sync.dma_start(
                        wsrc[:], d[f"WT{p}"][(i + u) * 128:(i + u + 1) * 128,
                                             oc * 512:(oc + 1) * 512])
                    nc.vector.tensor_tensor(
                        wt[i + u][:, oc * 512:(oc + 1) * 512], pw[u][:],
                        wsrc[:], ALU.add)

    # ================= phases =================
    def load_x_and_project(b, xt, wt):
        for i in range(NT):
            nc.sync.dma_start(
                xt[i][:], d["xT"][b, i * 128:(i + 1) * 128, :])
        for p, store in (("q", qT[b]), ("k", kT[b])):
            recon_wt(p, wt, "proj")
            for ot in range(NT):
                ps = [psum.tile([128, 512], f32, tag="proj", bufs=2,
                                name=f"ps_{p}{b}_{ot}_{i}")
                      for i in range(NSC)]
                for it in range(NT):
                    for sc in range(NSC):
                        nc.tensor.matmul(
                            ps[sc][:],
                            wt[it][:, ot * 128:(ot + 1) * 128],
                            xt[it][:, sc * 512:(sc + 1) * 512],
                            start=(it == 0), stop=(it == NT - 1),
                        )
                for sc in range(NSC):
                    nc.vector.tensor_scalar_add(
                        store[ot][:, sc * 512:(sc + 1) * 512],
                        ps[sc][:], bcol[p][:, ot:ot + 1])

        # ---- v in natural [s, o] layout with interleaved ones ----
        recon_wt("v", wt, "proj")
        for st in range(NT):
            grp = vs[b][st][:, 0:NH * (HD + 1)].rearrange(
                "p (h d) -> p h d", d=HD + 1)
            nc.vector.memset(grp[:, :, HD:HD + 1], 1.0)
            nc.vector.memset(vs[b][st][:, NH * (HD + 1):VW], 0.0)
            ps = [psum.tile([128, 512], f32, tag="proj", bufs=2,
                            name=f"ps_v{b}_{st}_{i}")
                  for i in range(NSC)]
            for it in range(NT):
                for oc in range(NSC):
                    nc.tensor.matmul(
                        ps[oc][:],
                        xt[it][:, st * 128:(st + 1) * 128],
                        wt[it][:, oc * 512:(oc + 1) * 512],
                        start=(it == 0), stop=(it == NT - 1),
                    )
            for oc in range(NSC):
                dst = grp[:, oc * 8:(oc + 1) * 8, 0:HD]
                nc.vector.tensor_tensor(
                    dst, ps[oc][:],
                    bb["v"][:, oc * 512:(oc + 1) * 512], ALU.add)

    def attention(b):
        for j in range(NH // 2):
            for sc in range(NSC):
                pcs = [psum.tile([128, 512], f32, tag="ctx", bufs=2,
                                 name=f"pc{b}_{j}_{sc}_{u}")
                       for u in range(2)]
                es = {}

                def scores_step(kt):
                    pss = [psum.tile([128, 512], f32, tag="att", bufs=2,
                                     name=f"pssc{b}_{j}_{sc}_{kt}_{u}")
                           for u in range(2)]
                    nc.tensor.matmul(
                        pss[0][:],
                        kT[b][j][0:64, kt * 128:(kt + 1) * 128],
                        qT[b][j][0:64, sc * 512:(sc + 1) * 512],
                        start=True, stop=True,
                    )
                    nc.tensor.matmul(
                        pss[1][:],
                        kT[b][j][64:128, kt * 128:(kt + 1) * 128],
                        qT[b][j][64:128, sc * 512:(sc + 1) * 512],
                        start=True, stop=True,
                    )
                    for u in range(2):
                        e = epool.tile([128, 512], bf16, tag="E", bufs=4,
                                       name=f"e{b}_{j}_{sc}_{kt}_{u}")
                        nc.scalar.activation(
                            e[:], pss[u][:], AF.Exp, scale=0.125)
                        es[kt, u] = e

                def ctx_step(kt):
                    for u in range(2):
                        h = 2 * j + u
                        nc.tensor.matmul(
                            pcs[u][:],
                            vs[b][kt][:, 65 * h:65 * h + 128],
                            es[kt, u][:],
                            start=(kt == 0), stop=(kt == NT - 1),
                        )

                # software pipeline: ctx matmuls trail scores by one kt
                scores_step(0)
                for kt in range(1, NT):
                    scores_step(kt)
                    ctx_step(kt - 1)
                ctx_step(NT - 1)

                for u in range(2):
                    hp = u * 64
                    srow = spool.tile([1, 512], f32, tag="srow", bufs=1,
                                      name=f"srow{b}_{j}_{sc}_{u}")
                    nc.vector.tensor_copy(srow[:], pcs[u][64:65, :])
                    recip = spool.tile([1, 512], f32, tag="recip", bufs=1,
                                       name=f"recip{b}_{j}_{sc}_{u}")
                    nc.vector.reciprocal_approx_fast(recip[:], srow[:])
                    rb = spool.tile([64, 512], f32, tag="rb", bufs=1,
                                    name=f"rb{b}_{j}_{sc}_{u}")
                    nc.gpsimd.partition_broadcast(rb[:], recip[:])
                    nc.vector.tensor_tensor(
                        cT[b][j][hp:hp + 64, sc * 512:(sc + 1) * 512],
                        pcs[u][0:64, :], rb[:], ALU.mult)

    def out_projection(b, wt):
        for st in range(NT):
            ps = [psum.tile([128, 512], f32, tag="oproj", bufs=2,
                            name=f"ps_o{b}_{st}_{i}")
                  for i in range(NSC)]
            for it in range(NT):
                for oc in range(NSC):
                    nc.tensor.matmul(
                        ps[oc][:],
                        cT[b][it][:, st * 128:(st + 1) * 128],
                        wt[it][:, oc * 512:(oc + 1) * 512],
                        start=(it == 0), stop=(it == NT - 1),
                    )
            for oc in range(NSC):
                osb = opool.tile([128, 512], f32, tag="osb", bufs=2,
                                 name=f"osb{b}_{st}_{oc}")
                nc.vector.tensor_tensor(
                    osb[:], ps[oc][:],
                    bb["o"][:, oc * 512:(oc + 1) * 512], ALU.add)
                nc.sync.dma_start(
                    d["out"][b, st * 128:(st + 1) * 128,
                             oc * 512:(oc + 1) * 512], osb[:])

    # ================= schedule =================
    # batch 0 projections; batch 0 attention; batch 1 projections are
    # emitted before batch 0's out-projection so the PE has projection
    # work while batch 0's ACT-bound attention runs. wt_o is
    # reconstructed once into batch 1's wt tiles (after its v
    # projection) and serves both out-projections.
    with tc.tile_pool(name="xw0", bufs=1) as pxw0:
        xt0 = [pxw0.tile([128, S], fp16, name=f"xt0_{i}")
               for i in range(NT)]
        wt0 = [pxw0.tile([128, H], fp16, name=f"wt0_{i}")
               for i in range(NT)]
        load_x_and_project(0, xt0, wt0)
    attention(0)
    with tc.tile_pool(name="xw1", bufs=1) as pxw1:
        xt1 = [pxw1.tile([128, S], fp16, name=f"xt1_{i}")
               for i in range(NT)]
        wt1 = [pxw1.tile([128, H], fp16, name=f"wt1_{i}")
               for i in range(NT)]
        load_x_and_project(1, xt1, wt1)
        recon_wt("o", wt1, "oproj")
        attention(1)
        out_projection(0, wt1)
        out_projection(1, wt1)


def build_nc():
    nc = bacc.Bacc("TRN2", target_bir_lowering=False, debug=False,
                   num_devices=N_CORES)
    d = {}
    d["xT"] = nc.dram_tensor("xT", [BPC, H, S], fp16,
                             kind="ExternalInput").ap()
    for p in PROJS:
        d[f"WT{p}"] = nc.dram_tensor(f"WT{p}", [H, H], fp16,
                                     kind="ExternalInput").ap()
        d[f"LMT{p}"] = nc.dram_tensor(f"LMT{p}", [128, H], fp16,
                                      kind="ExternalInput").ap()
        d[f"RM{p}"] = nc.dram_tensor(f"RM{p}", [128, H], fp16,
                                     kind="ExternalInput").ap()
    for p in ("q", "k"):
        d[f"BCOL{p}"] = nc.dram_tensor(f"BCOL{p}", [128, NT], f32,
                                       kind="ExternalInput").ap()
    for p in ("v", "o"):
        d[f"BROW{p}"] = nc.dram_tensor(f"BROW{p}", [1, H], f32,
                                       kind="ExternalInput").ap()
    d["out"] = nc.dram_tensor("out", [BPC, S, H], f32,
                              kind="ExternalOutput").ap()

    with tile.TileContext(nc) as tc, ExitStack() as ctx:
        _emit(ctx, tc, d)
    nc.compile()
    return nc


_CACHE = {}


def _prep_inputs(inputs):
    """Host-side prep: transposes + slicing per core + fp16 casts."""
    g = {k: np.asarray(v, dtype=np.float32) for k, v in inputs.items()
         if k != "task"}
    shared = {}
    for p in PROJS:
        WT = np.ascontiguousarray(g["W" + p].T)
        LMT = np.ascontiguousarray(g["LM" + p].T)
        F = g["F" + p]
        if not np.all(F == 1.0):
            # fold the per-output-channel SFG scale into the transposed
            # weights (identity in practice: F is spec'd all-ones)
            WT = WT * F
            LMT = np.ascontiguousarray(LMT * F)
        shared[f"WT{p}"] = WT.astype(np.float16)
        LMT16 = LMT.astype(np.float16)
        RM16 = g["RM" + p].astype(np.float16)
        shared[f"LMT{p}"] = np.ascontiguousarray(np.vstack([LMT16, LMT16]))
        shared[f"RM{p}"] = np.ascontiguousarray(np.vstack([RM16, RM16]))
    for p in ("q", "k"):
        shared[f"BCOL{p}"] = np.ascontiguousarray(
            (g["b" + p] * g["F" + p]).reshape(NT, 128).T)
    for p in ("v", "o"):
        shared[f"BROW{p}"] = np.ascontiguousarray(
            (g["b" + p] * g["F" + p]).reshape(1, H))
    hs = g["hidden_states"]
    in_maps = []
    for c in range(N_CORES):
        m = dict(shared)
        m["xT"] = np.ascontiguousarray(
            hs[c * BPC:(c + 1) * BPC].transpose(0, 2, 1)).astype(np.float16)
        in_maps.append(m)
    return in_maps


def kernel(**inputs):
    if "nc" not in _CACHE:
        _CACHE["nc"] = build_nc()
    nc = _CACHE["nc"]
    in_maps = _prep_inputs(inputs)
    res = run_bass_kernel_spmd(nc, in_maps, list(range(N_CORES)))
    return np.concatenate([r["out"] for r in res.results], axis=0)
